# revision 13
# baseline (speedup 1.0000x reference)
"""Chamfer distance kernel for Trainium2 (8 NeuronCores, SPMD).

Reference computation:
    p1 = pc1.reshape(-1, 3)  [N1=16384, 3]
    p2 = pc2.reshape(-1, 3)  [N2=16384, 3]
    d[i, j] = ||p1_i - p2_j||
    out = mean_j(min_i d[i,j]) + mean_i(min_j d[i,j])

Strategy (sorted-window candidate search + exact host certification):
  - Both clouds are sorted by x on the host. For 16384 standard-normal
    points the NN distance is ~0.007 while a +-256-rank window in sorted
    x order spans ~0.1+ in x, so the true NN of a point lies inside a
    WIN=512 window around its own sorted rank for all but a handful of
    points. Each 128-point block of one cloud gets one K=24 matmul
    against its window of the other cloud: [24,128].T @ [24,WIN] ->
    PSUM [128, WIN] holding exact-ish d2 (double-compensated bf16, abs
    err ~5e-7), then a single DVE min-reduction -> [128,1] row-min.
  - Both directions are computed this way (32 tiles/core total); pc2
    blocks 16c..16c+15 and pc1 blocks 16c..16c+15 live on core c, so
    every block's min is complete on its core: no accumulator, no
    partition-axis reduction, no cross-core combine.
  - The moving-side operands are padded with far-away points (x=1024)
    so every block uses the same window offsets (uniform SPMD kernel);
    pad distances are ~3e6, never the min.
  - Host certification makes the result exact: a windowed min d is
    provably the true min when d <= x-distance to the nearest excluded
    sorted neighbor. The ~100-300 points failing that certificate get
    their min recomputed exactly in numpy (negligible host work).
  - Four blocks share one 4-bank PSUM tile. Consumption is split so
    ScalarE and DVE drain in parallel: most groups are evacuated
    PSUM->SBUF fp16 by ScalarE (the x512 pre-scale keeps d2 in fp16
    normal range), then DVE runs two 2x pairwise-min folds and one
    short reduce; the rest are reduced by DVE straight from PSUM via a
    [128, (4, 512)] view.
  - Walrus accepts only one sem-wait per compute instruction; Tile
    emits more on recycled slots. _legalize_waits strips transitively
    implied same-engine waits and splits the rest onto injected NoOps.
"""

import os
import sys

import numpy as np

for _p in ("/opt/trn_rl_repo",):
    if os.path.isdir(_p) and _p not in sys.path:
        sys.path.append(_p)

import ml_dtypes

import concourse.bass as bass
import concourse.mybir as mybir
import concourse.tile as tile
from concourse.bass_utils import run_bass_kernel_spmd

BF16 = ml_dtypes.bfloat16

N_CORES = 8
N = 16384             # points per cloud
K = 24                # augmented contraction depth (compensated bf16)
WIN = 512             # candidate window (sorted ranks) per 128-block
HALF = WIN // 2
SPAN = 1920 + WIN     # per-core moving-side span: 15*128 + WIN
NBLK = 16             # 128-point blocks per core per direction
PAD = 1024.0          # far-point coordinate for window padding
GRP = 4               # blocks per PSUM tile / DVE reduce
SCALE = 512.0         # power-of-two scale keeping d2 in fp16 normal range
DIRECT_GROUPS = (0,)  # groups reduced DVE-direct from PSUM (rest via ACT)

TRACE = False         # test harness can flip this for profiled runs
LAST_RESULTS = None   # stashed BassKernelResults for the test harness

_NC_CACHE = None


def _build_nc():
    """Build the per-core Bass module (same NEFF on all 8 cores)."""
    nc = bass.Bass(trn_type="TRN2")

    # Packed input columns: [w2 2048 | m1 SPAN | w1 2048 | m2 SPAN]
    # w2/w1: weight-side rows of this core's own pc2/pc1 blocks.
    # m1/m2: moving-side rows of the padded pc1/pc2 candidate spans.
    seg = 2048 + SPAN
    inp = nc.dram_tensor("inp", [K, 2 * seg], mybir.dt.bfloat16,
                         kind="ExternalInput")
    # mout[:, bj]      = min_d2 for pc2 point 128*(16c+bj)+p  (dir 1)
    # mout[:, 16+bj]   = min_d2 for pc1 point 128*(16c+bj)+p  (dir 2)
    mout = nc.dram_tensor("mout", [128, 2 * NBLK], mybir.dt.float32,
                          kind="ExternalOutput")

    with tile.TileContext(nc) as tc:
        with (
            tc.tile_pool(name="ins", bufs=1) as ins_pool,
            tc.tile_pool(name="psum", bufs=2, space="PSUM") as psum_pool,
            tc.tile_pool(name="f16", bufs=3) as f16_pool,
            tc.tile_pool(name="outs", bufs=1) as out_pool,
        ):
            # One DMA per direction, issued on the two HWDGE queues (SP
            # + ACT) so they dispatch and stream concurrently.
            d1_sb = ins_pool.tile([K, seg], mybir.dt.bfloat16, tag="d1")
            d2_sb = ins_pool.tile([K, seg], mybir.dt.bfloat16, tag="d2")
            nc.sync.dma_start(d1_sb[:], inp[:, 0:seg])
            nc.scalar.dma_start(d2_sb[:], inp[:, seg:2 * seg])
            w2_sb, m1_sb = d1_sb[:, 0:2048], d1_sb[:, 2048:seg]
            w1_sb, m2_sb = d2_sb[:, 0:2048], d2_sb[:, 2048:seg]

            mo = out_pool.tile([128, 2 * NBLK], mybir.dt.float32, tag="mo")

            # 32 blocks (16 per direction), GRP per PSUM tile.
            for g in range(2 * NBLK // GRP):
                pt = psum_pool.tile([128, GRP * WIN], mybir.dt.float32,
                                    tag="ps")
                for k in range(GRP):
                    i = g * GRP + k
                    wsb, msb, bj = (
                        (w2_sb, m1_sb, i) if i < NBLK
                        else (w1_sb, m2_sb, i - NBLK))
                    nc.tensor.matmul(
                        pt[:, k * WIN:(k + 1) * WIN],
                        wsb[:, 128 * bj:128 * bj + 128],
                        msb[:, 128 * bj:128 * bj + WIN],
                        start=True, stop=True,
                    )
                if g in DIRECT_GROUPS:
                    # one DVE min-reduce covers all GRP blocks
                    nc.vector.tensor_reduce(
                        out=mo[:, g * GRP:(g + 1) * GRP],
                        in_=pt[:].rearrange("p (a b) -> p a b", a=GRP),
                        axis=mybir.AxisListType.X, op=mybir.AluOpType.min,
                    )
                else:
                    # ScalarE evacuates to fp16; DVE folds at 2x then
                    # reduces the last quarter at 1x.
                    h = f16_pool.tile([128, GRP * WIN], mybir.dt.float16,
                                      tag="f16")
                    nc.scalar.copy(h[:], pt[:])
                    h3 = h[:].rearrange("p (a b) -> p a b", a=GRP)
                    q1, q2 = WIN // 2, WIN // 4
                    nc.vector.tensor_tensor(
                        out=h3[:, :, 0:q1], in0=h3[:, :, 0:q1],
                        in1=h3[:, :, q1:WIN], op=mybir.AluOpType.min)
                    nc.vector.tensor_tensor(
                        out=h3[:, :, 0:q2], in0=h3[:, :, 0:q2],
                        in1=h3[:, :, q2:q1], op=mybir.AluOpType.min)
                    nc.vector.tensor_reduce(
                        out=mo[:, g * GRP:(g + 1) * GRP],
                        in_=h3[:, :, 0:q2],
                        axis=mybir.AxisListType.X, op=mybir.AluOpType.min,
                    )

            nc.sync.dma_start(mout[:], mo[:])

    _legalize_waits(nc)
    return nc


def _legalize_waits(nc):
    """Walrus's per-instruction structs carry at most one sem-wait, but
    Tile's sem assignment can emit several (slot-recycle WAR + input RAW).

    1. Same-engine waits are dropped when a cross-engine wait remains:
       engines execute in order and the cross-engine consumer they wait
       on transitively waited on those same-engine ticks.
    2. The kernel-tail Drain waits on every DMA queue + PE + DVE; all of
       it is transitively covered by the single output DMA.
    3. Any instruction still carrying N>1 waits gets N-1 same-engine
       NoOps injected right before it, one overflow wait each.
    """
    import concourse.mybir as mybir

    blocks = nc.m.functions[0].blocks

    # 1. same-engine strip
    for blk in blocks:
        for ins in blk.instructions:
            si = ins.sync_info
            if si is None or len(si.on_wait) <= 1 or not si.on_update:
                continue
            self_eng = si.on_update[0].ant_name.split("_")[0]
            keep = [w for w in si.on_wait
                    if w.ant_name.split("_")[0] != self_eng]
            if keep and len(keep) < len(si.on_wait):
                si.on_wait = keep
                ins.sync_info = si

    # 2. tail drain: keep only the output DMA queue's wait
    out_sems = set()
    for blk in blocks:
        for ins in blk.instructions:
            if type(ins).__name__ == "InstDMACopy" and ins.outs and \
                    getattr(ins.outs[0], "memref", "") == "mout":
                si = ins.sync_info
                for u in (si.on_update if si else []):
                    out_sems.add(u.ant_name)
    for blk in blocks:
        for ins in blk.instructions:
            if type(ins).__name__ != "InstDrain" or not out_sems:
                continue
            si = ins.sync_info
            if si is None or len(si.on_wait) <= 1:
                continue
            keep = [w for w in si.on_wait if w.ant_name in out_sems]
            if keep and len(keep) < len(si.on_wait):
                si.on_wait = keep
                ins.sync_info = si

    # 3. split remaining multi-waits onto same-engine NoOps
    eng_by_prefix = {
        "PE": mybir.EngineType.PE,
        "DVE": mybir.EngineType.DVE,
        "ACT": mybir.EngineType.Activation,
        "POOL": mybir.EngineType.Pool,
        "SP": mybir.EngineType.SP,
    }
    nop_id = [0]
    for blk in blocks:
        new_list = []
        changed = False
        for ins in blk.instructions:
            si = ins.sync_info
            if si is not None and len(si.on_wait) > 1:
                eng = getattr(ins, "engine", None)
                if eng is None and si.on_update:
                    eng = eng_by_prefix.get(
                        si.on_update[0].ant_name.split("_")[0])
                assert eng is not None, \
                    f"{ins.name}: cannot infer engine for wait split"
                waits = list(si.on_wait)
                for w in waits[:-1]:
                    nop_id[0] += 1
                    nop = mybir.InstNoOp(
                        name=f"I-waitnop-{nop_id[0]}", ins=[], outs=[],
                        engine=eng,
                        sync_info=mybir.SyncInfo(on_wait=[w], on_update=[]),
                    )
                    new_list.append(nop)
                si.on_wait = [waits[-1]]
                ins.sync_info = si
                changed = True
            new_list.append(ins)
        if changed:
            blk.instructions = new_list


def _split3(x):
    """fp32 -> three bf16 terms with x ~= h + m + l (residual ~2^-24 |x|)."""
    h = x.astype(BF16)
    r = x - h.astype(np.float32)
    m = r.astype(BF16)
    l = (r - m.astype(np.float32)).astype(BF16)
    return h, m, l


def _prep_side(p):
    """p: [N, 3] fp32 -> (weight_rows [24, N], moving_rows [24, N]).

    Row r of the weight side pairs with row r of the other cloud's moving
    side; the contraction sums, per coordinate, the six hi/mid/lo product
    terms of magnitude >= ~2^-17 (double-compensated bf16 dot, error
    ~2.5e-7), plus three hi/mid/lo rows for each side's |p|^2, so PSUM
    holds d2 = |w|^2 + |m|^2 - 2 w.m in nearly-fp32 precision.

    The weight side carries SCALE (a power of two), so PSUM holds
    SCALE*d2 -- keeping d2 mins inside fp16 normal range for the
    ScalarE-evacuated groups.
    """
    x, y, z = p[:, 0], p[:, 1], p[:, 2]
    sq = (x * x + y * y + z * z).astype(np.float32)
    w_rows, m_rows = [], []
    for c in (x, y, z):
        h, m, l = _split3(c)
        # (W, M) pairs: (h,h) (m,h) (h,m) (l,h) (m,m) (h,l)
        w_rows += [-2 * SCALE * h, -2 * SCALE * m, -2 * SCALE * h,
                   -2 * SCALE * l, -2 * SCALE * m, -2 * SCALE * h]
        m_rows += [h, h, m, h, m, l]
    ones = np.ones_like(sq)
    w_rows += [SCALE * ones] * 3 + list(_split3(SCALE * sq))
    m_rows += list(_split3(sq)) + [ones] * 3
    return (np.stack(w_rows).astype(BF16), np.stack(m_rows).astype(BF16))


def _exact_min_d2(q, ref):
    """Exact per-point min ||q_i - ref_j||^2 over all ref (host patch)."""
    d2 = ((q * q).sum(1)[:, None] + (ref * ref).sum(1)[None, :]
          - 2.0 * (q @ ref.T))
    return np.maximum(d2, 0.0).min(1)


def _certify_patch(d2_min, qs, refs):
    """Windowed mins -> exact mins.

    d2_min[j] is the min over sorted-ref ranks [128b+64-HALF, 128b+64+HALF)
    (clipped), b = j // 128. The min is provably exact when
    sqrt(d2) <= x-distance to the nearest excluded sorted ref point;
    everything else is recomputed exactly.
    """
    n = len(qs)
    j = np.arange(n)
    b = j // 128
    lo = 128 * b + 64 - HALF
    hi = 128 * b + 64 + HALF
    xq, xr = qs[:, 0], refs[:, 0]
    guard_lo = np.where(lo > 0, xq - xr[np.clip(lo - 1, 0, n - 1)], np.inf)
    guard_hi = np.where(hi < n, xr[np.clip(hi, 0, n - 1)] - xq, np.inf)
    guard = np.minimum(guard_lo, guard_hi)
    d = np.sqrt(np.maximum(d2_min, 0.0))
    fail = d > guard - 1e-6
    if fail.any():
        idx = np.where(fail)[0]
        d2_min = d2_min.copy()
        d2_min[idx] = _exact_min_d2(qs[idx], refs)
    return d2_min


def kernel(pc1, pc2):
    global _NC_CACHE, LAST_RESULTS
    p1 = np.ascontiguousarray(np.asarray(pc1, dtype=np.float32).reshape(-1, 3))
    p2 = np.ascontiguousarray(np.asarray(pc2, dtype=np.float32).reshape(-1, 3))
    assert p1.shape == (N, 3) and p2.shape == (N, 3)

    s1 = np.argsort(p1[:, 0], kind="stable")
    s2 = np.argsort(p2[:, 0], kind="stable")
    p1s, p2s = p1[s1], p2[s2]

    pad = np.full((HALF, 3), PAD, dtype=np.float32)
    p1pad = np.concatenate([pad, p1s, pad])
    p2pad = np.concatenate([pad, p2s, pad])

    w1, _ = _prep_side(p1s)
    w2, _ = _prep_side(p2s)
    _, m1 = _prep_side(p1pad)
    _, m2 = _prep_side(p2pad)

    in_maps = []
    for c in range(N_CORES):
        a = 2048 * c
        packed = np.concatenate(
            [w2[:, a:a + 2048], m1[:, a + 64:a + 64 + SPAN],
             w1[:, a:a + 2048], m2[:, a + 64:a + 64 + SPAN]], axis=1)
        in_maps.append({"inp": np.ascontiguousarray(packed)})

    if _NC_CACHE is None:
        _NC_CACHE = _build_nc()

    res = run_bass_kernel_spmd(
        _NC_CACHE, in_maps, core_ids=list(range(N_CORES)), trace=TRACE,
    )
    LAST_RESULTS = res

    d2_1 = np.concatenate([r["mout"][:, 0:NBLK].T.reshape(-1)
                           for r in res.results]) / SCALE   # sorted-pc2 order
    d2_2 = np.concatenate([r["mout"][:, NBLK:2 * NBLK].T.reshape(-1)
                           for r in res.results]) / SCALE   # sorted-pc1 order

    d2_1 = _certify_patch(d2_1, p2s, p1s)
    d2_2 = _certify_patch(d2_2, p1s, p2s)

    dist1 = np.sqrt(np.maximum(d2_1, 0.0))
    dist2 = np.sqrt(np.maximum(d2_2, 0.0))
    return np.asarray(dist1.mean() + dist2.mean(), dtype=np.float32)


# revision 20
# speedup vs baseline: 1.1292x; 1.1292x over previous
"""Chamfer distance kernel for Trainium2 (8 NeuronCores, SPMD).

Reference computation:
    p1 = pc1.reshape(-1, 3)  [N1=16384, 3]
    p2 = pc2.reshape(-1, 3)  [N2=16384, 3]
    d[i, j] = ||p1_i - p2_j||
    out = mean_j(min_i d[i,j]) + mean_i(min_j d[i,j])

Strategy (sorted-window candidate search + exact host certification):
  - Both clouds are sorted by x on the host. For 16384 standard-normal
    points the NN distance is ~0.007 while a +-256-rank window in sorted
    x order spans ~0.1+ in x, so the true NN of a point lies inside a
    WIN=512 window around its own sorted rank for all but a handful of
    points. Each 128-point block of one cloud gets one K=24 matmul
    against its window of the other cloud: [24,128].T @ [24,WIN] ->
    PSUM [128, WIN] holding exact-ish d2 (double-compensated bf16, abs
    err ~5e-7), then a single DVE min-reduction -> [128,1] row-min.
  - Both directions are computed this way (32 tiles/core total); pc2
    blocks 16c..16c+15 and pc1 blocks 16c..16c+15 live on core c, so
    every block's min is complete on its core: no accumulator, no
    partition-axis reduction, no cross-core combine.
  - The moving-side operands are padded with far-away points (x=1024)
    so every block uses the same window offsets (uniform SPMD kernel);
    pad distances are ~3e6, never the min.
  - Host certification makes the result exact: a windowed min d is
    provably the true min when d <= x-distance to the nearest excluded
    sorted neighbor. The ~100-300 points failing that certificate get
    their min recomputed exactly in numpy (negligible host work).
  - Four blocks share one 4-bank PSUM tile. Consumption is split so
    ScalarE and DVE drain in parallel: most groups are evacuated
    PSUM->SBUF fp16 by ScalarE (the x512 pre-scale keeps d2 in fp16
    normal range), then DVE runs two 2x pairwise-min folds and one
    short reduce; the rest are reduced by DVE straight from PSUM via a
    [128, (4, 512)] view.
  - Walrus accepts only one sem-wait per compute instruction; Tile
    emits more on recycled slots. _legalize_waits strips transitively
    implied same-engine waits and splits the rest onto injected NoOps.
"""

import os
import sys

import numpy as np

for _p in ("/opt/trn_rl_repo",):
    if os.path.isdir(_p) and _p not in sys.path:
        sys.path.append(_p)

import ml_dtypes

import concourse.bass as bass
import concourse.mybir as mybir
import concourse.tile as tile
from concourse.bass_utils import run_bass_kernel_spmd

BF16 = ml_dtypes.bfloat16

N_CORES = 8
N = 16384             # points per cloud
K = 24                # augmented contraction depth (compensated bf16)
WIN = 384             # candidate window (sorted ranks) per 128-block
SLOT = 512            # PSUM columns per block (bank-aligned matmul slot)
HALF = WIN // 2
SPAN = 1920 + WIN     # per-core moving-side span: 15*128 + WIN
NBLK = 16             # 128-point blocks per core per direction
WARMUP_MM = 8         # dummy matmuls during the input DMA (HAM un-throttle)
PAD = 1024.0          # far-point coordinate for window padding
GRP = 4               # blocks per PSUM tile / DVE reduce
SCALE = 512.0         # power-of-two scale keeping d2 in fp16 normal range
DIRECT_GROUPS = (0,)  # groups reduced DVE-direct from PSUM (rest via ACT)

TRACE = False         # test harness can flip this for profiled runs
LAST_RESULTS = None   # stashed BassKernelResults for the test harness

_NC_CACHE = None


def _build_nc():
    """Build the per-core Bass module (same NEFF on all 8 cores)."""
    nc = bass.Bass(trn_type="TRN2")

    # Packed input columns: [w2 2048 | m1 SPAN | w1 2048 | m2 SPAN]
    # w2/w1: weight-side rows of this core's own pc2/pc1 blocks.
    # m1/m2: moving-side rows of the padded pc1/pc2 candidate spans.
    seg = 2048 + SPAN
    inp = nc.dram_tensor("inp", [K, 2 * seg], mybir.dt.bfloat16,
                         kind="ExternalInput")
    # mout[:, bj]      = min_d2 for pc2 point 128*(16c+bj)+p  (dir 1)
    # mout[:, 16+bj]   = min_d2 for pc1 point 128*(16c+bj)+p  (dir 2)
    mout = nc.dram_tensor("mout", [128, 2 * NBLK], mybir.dt.float32,
                          kind="ExternalOutput")

    with tile.TileContext(nc) as tc:
        with (
            tc.tile_pool(name="ins", bufs=1) as ins_pool,
            tc.tile_pool(name="psum", bufs=2, space="PSUM") as psum_pool,
            tc.tile_pool(name="f16", bufs=3) as f16_pool,
            tc.tile_pool(name="outs", bufs=1) as out_pool,
        ):
            # Input DMAs: three on the SP queue rings (dir-1 operands,
            # w2 first since the first matmuls gate on it), dir-2 as one
            # DMA on the ACT queue -- all four stream concurrently.
            d1_sb = ins_pool.tile([K, seg], mybir.dt.bfloat16, tag="d1")
            d2_sb = ins_pool.tile([K, seg], mybir.dt.bfloat16, tag="d2")
            w2_sb, m1_sb = d1_sb[:, 0:2048], d1_sb[:, 2048:seg]
            w1_sb, m2_sb = d2_sb[:, 0:2048], d2_sb[:, 2048:seg]
            nc.sync.dma_start(d1_sb[:, 0:2048], inp[:, 0:2048])
            m_half = SPAN // 2
            nc.sync.dma_start(d1_sb[:, 2048:2048 + m_half],
                              inp[:, 2048:2048 + m_half])
            nc.sync.dma_start(d1_sb[:, 2048 + m_half:seg],
                              inp[:, 2048 + m_half:seg])
            nc.scalar.dma_start(d2_sb[:], inp[:, seg:2 * seg])

            # Keep PE busy while inputs stream: the HAM activity monitor
            # un-throttles the PE clock (1.2 -> 2.4 GHz) after ~3.4us of
            # sustained matmul activity, so every real matmul runs warm.
            wu_ps = psum_pool.tile([128, GRP * SLOT], mybir.dt.float32,
                                   tag="ps")
            garbage = out_pool.tile([K, SLOT], mybir.dt.bfloat16, tag="wg")
            nc.vector.memset(garbage[:], 1.0)
            for _ in range(WARMUP_MM):
                nc.tensor.matmul(wu_ps[:, 0:SLOT], garbage[:, 0:128],
                                 garbage[:], start=True, stop=True)

            mo = out_pool.tile([128, 2 * NBLK], mybir.dt.float32, tag="mo")

            # 32 blocks (16 per direction), GRP per PSUM tile; each
            # matmul lands in its own 512-col (bank-aligned) slot.
            for g in range(2 * NBLK // GRP):
                pt = psum_pool.tile([128, GRP * SLOT], mybir.dt.float32,
                                    tag="ps")
                pt3 = pt[:].rearrange("p (a b) -> p a b", a=GRP)
                for k in range(GRP):
                    i = g * GRP + k
                    wsb, msb, bj = (
                        (w2_sb, m1_sb, i) if i < NBLK
                        else (w1_sb, m2_sb, i - NBLK))
                    nc.tensor.matmul(
                        pt[:, k * SLOT:k * SLOT + WIN],
                        wsb[:, 128 * bj:128 * bj + 128],
                        msb[:, 128 * bj:128 * bj + WIN],
                        start=True, stop=True,
                    )
                if g in DIRECT_GROUPS:
                    # one DVE min-reduce covers all GRP blocks
                    nc.vector.tensor_reduce(
                        out=mo[:, g * GRP:(g + 1) * GRP],
                        in_=pt3[:, :, 0:WIN],
                        axis=mybir.AxisListType.X, op=mybir.AluOpType.min,
                    )
                else:
                    # ScalarE evacuates to fp16; DVE folds at 2x then
                    # reduces the last quarter at 1x.
                    h = f16_pool.tile([128, GRP * WIN], mybir.dt.float16,
                                      tag="f16")
                    h3 = h[:].rearrange("p (a b) -> p a b", a=GRP)
                    nc.scalar.copy(h3[:], pt3[:, :, 0:WIN])
                    q1, q2 = WIN // 2, WIN // 4
                    nc.vector.tensor_tensor(
                        out=h3[:, :, 0:q1], in0=h3[:, :, 0:q1],
                        in1=h3[:, :, q1:WIN], op=mybir.AluOpType.min)
                    nc.vector.tensor_tensor(
                        out=h3[:, :, 0:q2], in0=h3[:, :, 0:q2],
                        in1=h3[:, :, q2:q1], op=mybir.AluOpType.min)
                    nc.vector.tensor_reduce(
                        out=mo[:, g * GRP:(g + 1) * GRP],
                        in_=h3[:, :, 0:q2],
                        axis=mybir.AxisListType.X, op=mybir.AluOpType.min,
                    )

            nc.sync.dma_start(mout[:], mo[:])

    _legalize_waits(nc)
    return nc


def _legalize_waits(nc):
    """Walrus's per-instruction structs carry at most one sem-wait, but
    Tile's sem assignment can emit several (slot-recycle WAR + input RAW).

    1. Same-engine waits are dropped when a cross-engine wait remains:
       engines execute in order and the cross-engine consumer they wait
       on transitively waited on those same-engine ticks.
    2. The kernel-tail Drain waits on every DMA queue + PE + DVE; all of
       it is transitively covered by the single output DMA.
    3. Any instruction still carrying N>1 waits gets N-1 same-engine
       NoOps injected right before it, one overflow wait each.
    """
    import concourse.mybir as mybir

    blocks = nc.m.functions[0].blocks

    # 1. same-engine strip
    for blk in blocks:
        for ins in blk.instructions:
            si = ins.sync_info
            if si is None or len(si.on_wait) <= 1 or not si.on_update:
                continue
            self_eng = si.on_update[0].ant_name.split("_")[0]
            keep = [w for w in si.on_wait
                    if w.ant_name.split("_")[0] != self_eng]
            if keep and len(keep) < len(si.on_wait):
                si.on_wait = keep
                ins.sync_info = si

    # 2. tail drain: keep only the output DMA queue's wait
    out_sems = set()
    for blk in blocks:
        for ins in blk.instructions:
            if type(ins).__name__ == "InstDMACopy" and ins.outs and \
                    getattr(ins.outs[0], "memref", "") == "mout":
                si = ins.sync_info
                for u in (si.on_update if si else []):
                    out_sems.add(u.ant_name)
    for blk in blocks:
        for ins in blk.instructions:
            if type(ins).__name__ != "InstDrain" or not out_sems:
                continue
            si = ins.sync_info
            if si is None or len(si.on_wait) <= 1:
                continue
            keep = [w for w in si.on_wait if w.ant_name in out_sems]
            if keep and len(keep) < len(si.on_wait):
                si.on_wait = keep
                ins.sync_info = si

    # 3. split remaining multi-waits onto same-engine NoOps
    eng_by_prefix = {
        "PE": mybir.EngineType.PE,
        "DVE": mybir.EngineType.DVE,
        "ACT": mybir.EngineType.Activation,
        "POOL": mybir.EngineType.Pool,
        "SP": mybir.EngineType.SP,
    }
    nop_id = [0]
    for blk in blocks:
        new_list = []
        changed = False
        for ins in blk.instructions:
            si = ins.sync_info
            if si is not None and len(si.on_wait) > 1:
                eng = getattr(ins, "engine", None)
                if eng is None and si.on_update:
                    eng = eng_by_prefix.get(
                        si.on_update[0].ant_name.split("_")[0])
                assert eng is not None, \
                    f"{ins.name}: cannot infer engine for wait split"
                waits = list(si.on_wait)
                for w in waits[:-1]:
                    nop_id[0] += 1
                    nop = mybir.InstNoOp(
                        name=f"I-waitnop-{nop_id[0]}", ins=[], outs=[],
                        engine=eng,
                        sync_info=mybir.SyncInfo(on_wait=[w], on_update=[]),
                    )
                    new_list.append(nop)
                si.on_wait = [waits[-1]]
                ins.sync_info = si
                changed = True
            new_list.append(ins)
        if changed:
            blk.instructions = new_list


def _split3(x):
    """fp32 -> three bf16 terms with x ~= h + m + l (residual ~2^-24 |x|)."""
    h = x.astype(BF16)
    r = x - h.astype(np.float32)
    m = r.astype(BF16)
    l = (r - m.astype(np.float32)).astype(BF16)
    return h, m, l


def _prep_side(p):
    """p: [N, 3] fp32 -> (weight_rows [24, N], moving_rows [24, N]).

    Row r of the weight side pairs with row r of the other cloud's moving
    side; the contraction sums, per coordinate, the six hi/mid/lo product
    terms of magnitude >= ~2^-17 (double-compensated bf16 dot, error
    ~2.5e-7), plus three hi/mid/lo rows for each side's |p|^2, so PSUM
    holds d2 = |w|^2 + |m|^2 - 2 w.m in nearly-fp32 precision.

    The weight side carries SCALE (a power of two), so PSUM holds
    SCALE*d2 -- keeping d2 mins inside fp16 normal range for the
    ScalarE-evacuated groups.
    """
    x, y, z = p[:, 0], p[:, 1], p[:, 2]
    sq = (x * x + y * y + z * z).astype(np.float32)
    w_rows, m_rows = [], []
    for c in (x, y, z):
        h, m, l = _split3(c)
        # (W, M) pairs: (h,h) (m,h) (h,m) (l,h) (m,m) (h,l)
        w_rows += [-2 * SCALE * h, -2 * SCALE * m, -2 * SCALE * h,
                   -2 * SCALE * l, -2 * SCALE * m, -2 * SCALE * h]
        m_rows += [h, h, m, h, m, l]
    ones = np.ones_like(sq)
    w_rows += [SCALE * ones] * 3 + list(_split3(SCALE * sq))
    m_rows += list(_split3(sq)) + [ones] * 3
    return (np.stack(w_rows).astype(BF16), np.stack(m_rows).astype(BF16))


def _exact_min_d2(q, ref):
    """Exact per-point min ||q_i - ref_j||^2 over all ref (host patch)."""
    d2 = ((q * q).sum(1)[:, None] + (ref * ref).sum(1)[None, :]
          - 2.0 * (q @ ref.T))
    return np.maximum(d2, 0.0).min(1)


def _certify_patch(d2_min, qs, refs):
    """Windowed mins -> exact mins.

    d2_min[j] is the min over sorted-ref ranks [128b+64-HALF, 128b+64+HALF)
    (clipped), b = j // 128. The min is provably exact when
    sqrt(d2) <= x-distance to the nearest excluded sorted ref point;
    everything else is recomputed exactly.
    """
    n = len(qs)
    j = np.arange(n)
    b = j // 128
    lo = 128 * b + 64 - HALF
    hi = 128 * b + 64 + HALF
    xq, xr = qs[:, 0], refs[:, 0]
    guard_lo = np.where(lo > 0, xq - xr[np.clip(lo - 1, 0, n - 1)], np.inf)
    guard_hi = np.where(hi < n, xr[np.clip(hi, 0, n - 1)] - xq, np.inf)
    guard = np.minimum(guard_lo, guard_hi)
    d = np.sqrt(np.maximum(d2_min, 0.0))
    fail = d > guard - 1e-6
    if fail.any():
        idx = np.where(fail)[0]
        d2_min = d2_min.copy()
        d2_min[idx] = _exact_min_d2(qs[idx], refs)
    return d2_min


def kernel(pc1, pc2):
    global _NC_CACHE, LAST_RESULTS
    p1 = np.ascontiguousarray(np.asarray(pc1, dtype=np.float32).reshape(-1, 3))
    p2 = np.ascontiguousarray(np.asarray(pc2, dtype=np.float32).reshape(-1, 3))
    assert p1.shape == (N, 3) and p2.shape == (N, 3)

    s1 = np.argsort(p1[:, 0], kind="stable")
    s2 = np.argsort(p2[:, 0], kind="stable")
    p1s, p2s = p1[s1], p2[s2]

    pad = np.full((HALF, 3), PAD, dtype=np.float32)
    p1pad = np.concatenate([pad, p1s, pad])
    p2pad = np.concatenate([pad, p2s, pad])

    w1, _ = _prep_side(p1s)
    w2, _ = _prep_side(p2s)
    _, m1 = _prep_side(p1pad)
    _, m2 = _prep_side(p2pad)

    in_maps = []
    for c in range(N_CORES):
        a = 2048 * c
        packed = np.concatenate(
            [w2[:, a:a + 2048], m1[:, a + 64:a + 64 + SPAN],
             w1[:, a:a + 2048], m2[:, a + 64:a + 64 + SPAN]], axis=1)
        in_maps.append({"inp": np.ascontiguousarray(packed)})

    if _NC_CACHE is None:
        _NC_CACHE = _build_nc()

    res = run_bass_kernel_spmd(
        _NC_CACHE, in_maps, core_ids=list(range(N_CORES)), trace=TRACE,
    )
    LAST_RESULTS = res

    d2_1 = np.concatenate([r["mout"][:, 0:NBLK].T.reshape(-1)
                           for r in res.results]) / SCALE   # sorted-pc2 order
    d2_2 = np.concatenate([r["mout"][:, NBLK:2 * NBLK].T.reshape(-1)
                           for r in res.results]) / SCALE   # sorted-pc1 order

    d2_1 = _certify_patch(d2_1, p2s, p1s)
    d2_2 = _certify_patch(d2_2, p1s, p2s)

    dist1 = np.sqrt(np.maximum(d2_1, 0.0))
    dist2 = np.sqrt(np.maximum(d2_2, 0.0))
    return np.asarray(dist1.mean() + dist2.mean(), dtype=np.float32)
